# revision 2
# baseline (speedup 1.0000x reference)
"""ConvDeepSet kernel for Trainium2 (8 NeuronCores, Bass/Tile).

Math (per batch b, target point o, channel c):
    agg[o,c] = sum_i yd[i,c] * exp(-alpha_c * (x_i - t_o)^2)      yd = [1 | y]
    out[o,:] = [agg0, agg1/(agg0+eps), ...] @ W + b

Channels sharing a sigma value share the RBF matrix E[i,o], so with G
sigma-groups the aggregation collapses to G matmuls after folding W into
the context values on the host:
    U_g = sum_{c in g, c>0} yd[:,c] W[c,:]              (n_in, 16)
    P[o, 0]    = den[o]  = sum_i E_g0[i,o]              (density group g0)
    P[o, 1+j] += sum_i E_g[i,o] U_g[i,j]                (all groups)
    out[o,:]  = den*W[0,:] + P[o,1:]/(den+eps) + b

The exponent is a rank-3 matmul: -a(x-t)^2 = x*(2at) + x^2*(-a) + 1*(-at^2),
computed on the TensorEngine straight into PSUM; ScalarE applies exp.

Sharding: core c -> (batch c//2, output half c%2). Everything per-core fits
in SBUF; the (n_in, n_out, C) intermediate never materializes.
"""

import numpy as np

B, N_IN, N_OUT = 4, 1024, 1024
IN_CH_RAW, OUT_CH = 7, 16
IN_CH = IN_CH_RAW + 1
N_CORES = 8
O_CORE = N_OUT // 2          # 512 target points per core
P = 128                      # partitions
KI = N_IN // P               # 8 contraction chunks
KJ = O_CORE // P             # 4 output chunks
EPS = 1e-8

_BASS_CACHE: dict = {}


def _build(widths):
    """Build + compile the SPMD Bass program for group widths `widths`.

    widths[g] = columns of group g's aggregation rhs (17 for the group
    holding the density channel 0, 16 otherwise). Group 0 must be the
    density group.
    """
    import concourse.bacc as bacc
    import concourse.tile as tile
    from concourse import mybir

    f32 = mybir.dt.float32
    G = len(widths)
    wtot = sum(widths)
    offs = np.cumsum([0] + list(widths))  # per-group col offset into ydt blocks

    nc = bacc.Bacc("TRN2", target_bir_lowering=False, debug=False)

    lx_d = nc.dram_tensor("lx", [3, N_IN], f32, kind="ExternalInput")
    rt_d = nc.dram_tensor("rt", [3 * G, O_CORE], f32, kind="ExternalInput")
    ydt_d = nc.dram_tensor("ydt", [P, KI * wtot], f32, kind="ExternalInput")
    wb0_d = nc.dram_tensor("wb0", [P, OUT_CH], f32, kind="ExternalInput")
    bt_d = nc.dram_tensor("bt", [P, OUT_CH], f32, kind="ExternalInput")
    out_d = nc.dram_tensor("out", [O_CORE, OUT_CH], f32, kind="ExternalOutput")

    with tile.TileContext(nc) as tc:
        with (
            tc.tile_pool(name="const", bufs=1) as cpool,
            tc.tile_pool(name="epool", bufs=1) as epool,
            tc.tile_pool(name="small", bufs=2) as spool,
            tc.tile_pool(name="outp", bufs=2) as opool,
            tc.tile_pool(name="apsum", bufs=3, space="PSUM") as apsum,
            tc.tile_pool(name="ppsum", bufs=1, space="PSUM") as ppsum,
        ):
            lx = cpool.tile([3, N_IN], f32, tag="lx")
            nc.sync.dma_start(lx[:], lx_d[:])
            rt = cpool.tile([3 * G, O_CORE], f32, tag="rt")
            nc.sync.dma_start(rt[:], rt_d[:])
            ydt = cpool.tile([P, KI * wtot], f32, tag="ydt")
            nc.sync.dma_start(ydt[:], ydt_d[:])
            wb0 = cpool.tile([P, OUT_CH], f32, tag="wb0")
            nc.sync.dma_start(wb0[:], wb0_d[:])
            bt = cpool.tile([P, OUT_CH], f32, tag="bt")
            nc.sync.dma_start(bt[:], bt_d[:])

            # Phase 1: E_g,ki = exp(-a_g (x - t)^2) tiles, all resident in SBUF.
            E = {}
            for g in range(G):
                for ki in range(KI):
                    a_ps = apsum.tile([P, O_CORE], f32, tag="A")
                    nc.tensor.matmul(
                        a_ps[:],
                        lx[:, ki * P:(ki + 1) * P],
                        rt[3 * g:3 * g + 3, :],
                        start=True,
                        stop=True,
                    )
                    e = epool.tile([P, O_CORE], f32, tag=f"E{g}_{ki}")
                    nc.scalar.activation(e[:], a_ps[:], mybir.ActivationFunctionType.Exp)
                    E[(g, ki)] = e

            # Phase 2: P[kj] accumulates den (col 0) and V (cols 1:17) in PSUM.
            pacc = [
                ppsum.tile([P, 1 + OUT_CH], f32, tag=f"P{kj}", name=f"pacc{kj}")
                for kj in range(KJ)
            ]
            n_mm = G * KI
            for g in range(G):
                w = widths[g]
                coff = 0 if g == 0 else 1
                for ki in range(KI):
                    rhs = ydt[:, KI * offs[g] + ki * w: KI * offs[g] + (ki + 1) * w]
                    for kj in range(KJ):
                        idx = g * KI + ki
                        nc.tensor.matmul(
                            pacc[kj][:, coff:coff + w],
                            E[(g, ki)][:, kj * P:(kj + 1) * P],
                            rhs,
                            start=(idx == 0),
                            stop=(idx == n_mm - 1),
                            skip_group_check=(G > 1),
                        )

            # Phase 3: normalize + output.  out = den*W0 + V/(den+eps) + b
            for kj in range(KJ):
                denp = spool.tile([P, 1], f32, tag="denp")
                nc.vector.tensor_scalar_add(denp[:], pacc[kj][:, 0:1], EPS)
                recip = spool.tile([P, 1], f32, tag="recip")
                nc.vector.reciprocal(recip[:], denp[:])
                t1 = spool.tile([P, OUT_CH], f32, tag="t1")
                nc.vector.scalar_tensor_tensor(
                    t1[:], wb0[:], denp[:], bt[:],
                    op0=mybir.AluOpType.mult, op1=mybir.AluOpType.add,
                )
                o_sb = opool.tile([P, OUT_CH], f32, tag="osb")
                nc.vector.scalar_tensor_tensor(
                    o_sb[:], pacc[kj][:, 1:1 + OUT_CH], recip[:], t1[:],
                    op0=mybir.AluOpType.mult, op1=mybir.AluOpType.add,
                )
                nc.sync.dma_start(out_d[kj * P:(kj + 1) * P, :], o_sb[:])

    nc.compile()
    return nc


def _prepare_inputs(context_x, context_y, t, sigma, W, b):
    """Host-side prep: group channels by sigma, fold W, shard per core."""
    sigma = np.asarray(sigma, dtype=np.float32)
    W64 = np.asarray(W, dtype=np.float64)
    b64 = np.asarray(b, dtype=np.float64)

    # Group channels by exact sigma value; density group (channel 0) first.
    uniq = []
    for c in range(IN_CH):
        if sigma[c] not in uniq:
            uniq.append(sigma[c])
    uniq.sort(key=lambda s: (s != sigma[0]))  # group of channel 0 first
    groups = [[c for c in range(IN_CH) if sigma[c] == s] for s in uniq]
    alphas = [0.5 / np.exp(2.0 * np.float64(s)) for s in uniq]
    widths = tuple((1 + OUT_CH) if 0 in g else OUT_CH for g in groups)
    G = len(groups)

    in_maps = []
    for core in range(N_CORES):
        bidx, half = core // 2, core % 2
        x = np.asarray(context_x[bidx, :, 0], dtype=np.float64)
        th = np.asarray(t[bidx, half * O_CORE:(half + 1) * O_CORE, 0], dtype=np.float64)
        y = np.asarray(context_y[bidx], dtype=np.float64)  # (N_IN, 7)

        lx = np.stack([x, x * x, np.ones_like(x)]).astype(np.float32)  # (3, N_IN)

        rt = np.empty((3 * G, O_CORE), dtype=np.float32)
        for g, a in enumerate(alphas):
            rt[3 * g + 0] = 2.0 * a * th
            rt[3 * g + 1] = -a
            rt[3 * g + 2] = -a * th * th

        blocks = []
        for g, chans in enumerate(groups):
            w = widths[g]
            rhs = np.zeros((N_IN, w), dtype=np.float64)
            coff = 0
            if 0 in chans:
                rhs[:, 0] = 1.0
                coff = 1
            conv_ch = [c for c in chans if c > 0]
            if conv_ch:
                # yd[:, c] for c>0 is context_y[:, c-1]
                rhs[:, coff:] = y[:, [c - 1 for c in conv_ch]] @ W64[conv_ch, :]
            # chunk-major layout: ydt[p, ki*w + n] = rhs[ki*128 + p, n]
            blocks.append(
                rhs.reshape(KI, P, w).transpose(1, 0, 2).reshape(P, KI * w)
            )
        ydt = np.concatenate(blocks, axis=1).astype(np.float32)

        in_maps.append({
            "lx": lx,
            "rt": rt,
            "ydt": ydt,
            "wb0": np.tile(W64[0].astype(np.float32), (P, 1)),
            "bt": np.tile(b64.astype(np.float32), (P, 1)),
        })
    return widths, in_maps


def _run(inputs: dict, trace: bool = False):
    """Compile (cached), run on 8 cores, gather. Returns (output, results)."""
    from concourse.bass_utils import run_bass_kernel_spmd

    widths, in_maps = _prepare_inputs(
        inputs["context_x"], inputs["context_y"], inputs["t"],
        inputs["sigma"], inputs["W"], inputs["b"],
    )
    if widths not in _BASS_CACHE:
        _BASS_CACHE[widths] = _build(widths)
    nc = _BASS_CACHE[widths]

    res = run_bass_kernel_spmd(nc, in_maps, list(range(N_CORES)), trace=trace)

    out = np.empty((B, N_OUT, OUT_CH), dtype=np.float32)
    for core in range(N_CORES):
        bidx, half = core // 2, core % 2
        out[bidx, half * O_CORE:(half + 1) * O_CORE, :] = res.results[core]["out"]
    return out, res


def kernel(**inputs) -> np.ndarray:
    out, _ = _run(inputs, trace=False)
    return out


# revision 16
# speedup vs baseline: 1.9705x; 1.9705x over previous
"""ConvDeepSet kernel for Trainium2 (8 NeuronCores, Bass/Tile).

Math (per batch b, target point o, channel c):
    agg[o,c] = sum_i yd[i,c] * exp(-alpha_c * (x_i - t_o)^2)      yd = [1 | y]
    out[o,:] = [agg0, agg1/(agg0+eps), ...] @ W + b

Channels sharing a sigma value share the RBF matrix E[i,o], so with G
sigma-groups the aggregation collapses to G matmuls after folding W into
the context values on the host:
    U_g = sum_{c in g, c>0} yd[:,c] W[c,:]              (n_in, 16)
    P[o, 0]    = den[o]  = sum_i E_g0[i,o]              (density group g0)
    P[o, 1+j] += sum_i E_g[i,o] U_g[i,j]                (all groups)
    out[o,:]  = den*W[0,:] + P[o,1:]/(den+eps) + b

The exponent -a(x-t)^2 = s*u - s^2/2 - u^2/2 with s = sqrt(2a)x, u =
sqrt(2a)t is a rank-8 fp16 matmul using 2-way fp16 splits of s, u and of
q = s^2/2, w = u^2/2 (host-prepared), exact to ~1e-4 absolute — fp32
quality at fp16 matmul speed (single pass + fast weight load). ScalarE
applies exp (PSUM -> SBUF, fp16 out); the aggregation runs in fp16 with
fp32 PSUM accumulation (max abs output error ~5e-4 on the reference data,
~5e-5 of the output scale).

Sharding: core c -> (batch c//2, output half c%2). Per-core data all
lives in SBUF; the (n_in, n_out, C) intermediate never materializes.
"""

import numpy as np

B, N_IN, N_OUT = 4, 1024, 1024
IN_CH_RAW, OUT_CH = 7, 16
IN_CH = IN_CH_RAW + 1
N_CORES = 8
O_CORE = N_OUT // 2          # 512 target points per core
P = 128                      # partitions
KI = N_IN // P               # 8 contraction chunks
NPAIR = KI // 2              # exp processed in chunk pairs
KJ = O_CORE // P             # 4 output chunks
KEXP = 8                     # rows of the split-product exponent matmul
EPS = 1e-8

_BASS_CACHE: dict = {}


def _build_fp16_raw(widths):
    """Raw-Bass (no Tile) version of the fp16 pipeline: hand-rolled
    semaphores, no Tile entry/exit barrier stages — saves ~10us of
    framework overhead at these kernel sizes."""
    import concourse.bass as bass
    from concourse import mybir

    f32 = mybir.dt.float32
    f16 = mybir.dt.float16
    G = len(widths)
    wtot = sum(widths)
    offs = np.cumsum([0] + list(widths))
    npair = NPAIR * G
    Exp = mybir.ActivationFunctionType.Exp

    nc = bass.Bass("TRN2", target_bir_lowering=False, debug=False)

    # xr = per-group [lxh | rth] blocks along the free dim (groups must sit at
    # partition 0 for PE); one DMA feeds the first matmul. wbb = [wb0 | bt].
    BLK = N_IN + O_CORE
    xr_d = nc.dram_tensor("xr", [KEXP, G * BLK], f16, kind="ExternalInput")
    ydt_d = nc.dram_tensor("ydt", [P, KI * wtot], f16, kind="ExternalInput")
    wbb_d = nc.dram_tensor("wbb", [P, 2 * OUT_CH], f32, kind="ExternalInput")
    out_d = nc.dram_tensor("out", [O_CORE, OUT_CH], f32, kind="ExternalOutput")

    xr = nc.alloc_sbuf_tensor("xr_sb", [KEXP, G * BLK], f16).ap()
    lxh = [xr[:, g * BLK:g * BLK + N_IN] for g in range(G)]
    rth = [xr[:, g * BLK + N_IN:(g + 1) * BLK] for g in range(G)]
    ydt = nc.alloc_sbuf_tensor("ydt_sb", [P, KI * wtot], f16).ap()
    wbb = nc.alloc_sbuf_tensor("wbb_sb", [P, 2 * OUT_CH], f32).ap()
    wb0 = wbb[:, :OUT_CH]
    bt = wbb[:, OUT_CH:]
    warm = nc.alloc_sbuf_tensor("warm_sb", [1, 1], f32).ap()
    E = [nc.alloc_sbuf_tensor(f"e_sb{q}", [P, 2 * O_CORE], f16).ap()
         for q in range(npair)]
    denp = [nc.alloc_sbuf_tensor(f"denp_sb{kj}", [P, 1], f32).ap()
            for kj in range(KJ)]
    recip = [nc.alloc_sbuf_tensor(f"recip_sb{kj}", [P, 1], f32).ap()
             for kj in range(KJ)]
    t1 = [nc.alloc_sbuf_tensor(f"t1_sb{kj}", [P, OUT_CH], f32).ap()
          for kj in range(KJ)]
    o_all = nc.alloc_sbuf_tensor("o_all_sb", [P, KJ * OUT_CH], f32).ap()
    o_sb = [o_all[:, kj * OUT_CH:(kj + 1) * OUT_CH] for kj in range(KJ)]
    A = [nc.alloc_psum_tensor(f"a_ps{i}", [P, 2 * O_CORE], f32).ap()
         for i in range(2)]
    pacc = [nc.alloc_psum_tensor(f"pacc{kj}", [P, 1 + OUT_CH], f32).ap()
            for kj in range(KJ)]

    with (
        nc.Block() as block,
        nc.semaphore("dsem_s") as dsem_s,   # sync-queue input DMAs
        nc.semaphore("dsem_g") as dsem_g,   # gpsimd-queue input DMAs
        nc.semaphore("psem") as psem,       # PE exponent matmuls done
        nc.semaphore("asem") as asem,       # ACT exp pairs done
        nc.semaphore("gsem") as gsem,       # PE agg per-kj done
        nc.semaphore("vsem") as vsem,       # DVE epilogue per-kj done
        nc.semaphore("vv") as vv,           # DVE same-engine pipeline sync
        nc.semaphore("osem") as osem,       # out DMAs done
    ):
        @block.sync
        def _(sync):
            sync.dma_start(out=xr[:], in_=xr_d[:]).then_inc(dsem_s, 16)
            sync.wait_ge(vsem, KJ)
            # out viewed as (KJ, P, OUT_CH) <- o_all [P, (KJ, OUT_CH)]
            sync.dma_start(
                out=out_d.rearrange("(k p) j -> p k j", p=P),
                in_=o_all[:],
            ).then_inc(osem, 16)
            sync.wait_ge(osem, 16)

        @block.gpsimd
        def _(gpsimd):
            gpsimd.dma_start(out=ydt[:], in_=ydt_d[:]).then_inc(dsem_g, 16)
            gpsimd.dma_start(out=wbb[:], in_=wbb_d[:]).then_inc(dsem_g, 16)

        @block.tensor
        def _(tensor):
            tensor.wait_ge(dsem_s, 16)
            for q in range(npair):
                g, p = divmod(q, NPAIR)
                if q >= 2:
                    tensor.wait_ge(asem, q - 1)  # A buffer q%2 free again
                for half in range(2):
                    ki = 2 * p + half
                    tensor.matmul(
                        A[q % 2][:, half * O_CORE:(half + 1) * O_CORE],
                        lxh[g][:, ki * P:(ki + 1) * P],
                        rth[g][:],
                        start=True,
                        stop=True,
                    ).then_inc(psem, 1)
            tensor.wait_ge(dsem_g, 32)  # ydt (all gpsimd-queue DMAs)
            n_mm = G * KI
            for kj in range(KJ):
                for g in range(G):
                    w = widths[g]
                    coff = 0 if g == 0 else 1
                    for ki in range(KI):
                        idx = g * KI + ki
                        q = g * NPAIR + ki // 2
                        if kj == 0 and ki % 2 == 0:
                            tensor.wait_ge(asem, q + 1)
                        rhs = ydt[:, KI * offs[g] + ki * w:
                                  KI * offs[g] + (ki + 1) * w]
                        lhs = E[q][:, (ki % 2) * O_CORE + kj * P:
                                   (ki % 2) * O_CORE + (kj + 1) * P]
                        mm = tensor.matmul(
                            pacc[kj][:, coff:coff + w],
                            lhs,
                            rhs,
                            start=(idx == 0),
                            stop=(idx == n_mm - 1),
                            skip_group_check=(G > 1),
                        )
                        if idx == n_mm - 1:
                            mm.then_inc(gsem, 1)

        @block.scalar
        def _(scalar):
            # touch Exp before the pipeline needs it: loads the ACT table
            # while the input DMAs are still in flight
            scalar.activation(warm[:], nc.const_aps.tensor(0.0, (1, 1)), Exp)
            for q in range(npair):
                scalar.wait_ge(psem, 2 * (q + 1))
                scalar.activation(E[q][:], A[q % 2][:], Exp).then_inc(asem, 1)

        @block.vector
        def _(vector):
            vector.wait_ge(dsem_g, 32)  # wbb resident
            for kj in range(KJ):
                vector.wait_ge(gsem, kj + 1)
                vector.tensor_scalar_add(
                    denp[kj][:], pacc[kj][:, 0:1], EPS
                ).then_inc(vv, 1)
            vector.wait_ge(vv, KJ)  # denp writes through the pipe
            for kj in range(KJ):
                vector.scalar_tensor_tensor(
                    t1[kj][:], wb0[:], denp[kj][:], bt[:],
                    op0=mybir.AluOpType.mult, op1=mybir.AluOpType.add,
                ).then_inc(vv, 1)
                vector.reciprocal(recip[kj][:], denp[kj][:]).then_inc(vv, 1)
            vector.wait_ge(vv, 3 * KJ)  # t1 + recip through the pipe
            for kj in range(KJ):
                vector.scalar_tensor_tensor(
                    o_sb[kj][:], pacc[kj][:, 1:1 + OUT_CH], recip[kj][:], t1[kj][:],
                    op0=mybir.AluOpType.mult, op1=mybir.AluOpType.add,
                ).then_inc(vsem, 1)

    return nc


def _build_fp32(widths):
    """Fallback: fp32 rank-3 exponent matmul + fp32 aggregation (slower,
    used only when fp16 split values would overflow)."""
    import concourse.bacc as bacc
    import concourse.tile as tile
    from concourse import mybir

    f32 = mybir.dt.float32
    G = len(widths)
    wtot = sum(widths)
    offs = np.cumsum([0] + list(widths))

    nc = bacc.Bacc("TRN2", target_bir_lowering=False, debug=False)

    lx_d = nc.dram_tensor("lx", [3, N_IN], f32, kind="ExternalInput")
    rt_d = nc.dram_tensor("rt", [3 * G, O_CORE], f32, kind="ExternalInput")
    ydt_d = nc.dram_tensor("ydt", [P, KI * wtot], f32, kind="ExternalInput")
    wb0_d = nc.dram_tensor("wb0", [P, OUT_CH], f32, kind="ExternalInput")
    bt_d = nc.dram_tensor("bt", [P, OUT_CH], f32, kind="ExternalInput")
    out_d = nc.dram_tensor("out", [O_CORE, OUT_CH], f32, kind="ExternalOutput")

    with tile.TileContext(nc) as tc:
        with (
            tc.tile_pool(name="const", bufs=1) as cpool,
            tc.tile_pool(name="epool", bufs=1) as epool,
            tc.tile_pool(name="small", bufs=2) as spool,
            tc.tile_pool(name="outp", bufs=2) as opool,
            tc.tile_pool(name="apsum", bufs=3, space="PSUM") as apsum,
            tc.tile_pool(name="ppsum", bufs=1, space="PSUM") as ppsum,
        ):
            lx = cpool.tile([3, N_IN], f32, tag="lx")
            nc.sync.dma_start(lx[:], lx_d[:])
            rt = cpool.tile([3 * G, O_CORE], f32, tag="rt")
            nc.scalar.dma_start(rt[:], rt_d[:])
            ydt = cpool.tile([P, KI * wtot], f32, tag="ydt")
            nc.gpsimd.dma_start(ydt[:], ydt_d[:])
            wb0 = cpool.tile([P, OUT_CH], f32, tag="wb0")
            nc.gpsimd.dma_start(wb0[:], wb0_d[:])
            bt = cpool.tile([P, OUT_CH], f32, tag="bt")
            nc.gpsimd.dma_start(bt[:], bt_d[:])

            E = {}
            for g in range(G):
                for ki in range(KI):
                    a_ps = apsum.tile([P, O_CORE], f32, tag="A", name="a_ps")
                    nc.tensor.matmul(
                        a_ps[:],
                        lx[:, ki * P:(ki + 1) * P],
                        rt[3 * g:3 * g + 3, :],
                        start=True,
                        stop=True,
                    )
                    e = epool.tile([P, O_CORE], f32, tag=f"E{g}_{ki}", name="e")
                    nc.scalar.activation(
                        e[:], a_ps[:], mybir.ActivationFunctionType.Exp
                    )
                    E[(g, ki)] = e

            pacc = [
                ppsum.tile([P, 1 + OUT_CH], f32, tag=f"P{kj}", name=f"pacc{kj}")
                for kj in range(KJ)
            ]
            n_mm = G * KI
            for kj in range(KJ):
                for g in range(G):
                    w = widths[g]
                    coff = 0 if g == 0 else 1
                    for ki in range(KI):
                        idx = g * KI + ki
                        rhs = ydt[:, KI * offs[g] + ki * w: KI * offs[g] + (ki + 1) * w]
                        nc.tensor.matmul(
                            pacc[kj][:, coff:coff + w],
                            E[(g, ki)][:, kj * P:(kj + 1) * P],
                            rhs,
                            start=(idx == 0),
                            stop=(idx == n_mm - 1),
                            skip_group_check=(G > 1),
                        )

                denp = spool.tile([P, 1], f32, tag="denp", name="denp")
                nc.vector.tensor_scalar_add(denp[:], pacc[kj][:, 0:1], EPS)
                recip = spool.tile([P, 1], f32, tag="recip", name="recip")
                nc.vector.reciprocal(recip[:], denp[:])
                t1 = spool.tile([P, OUT_CH], f32, tag="t1", name="t1")
                nc.vector.scalar_tensor_tensor(
                    t1[:], wb0[:], denp[:], bt[:],
                    op0=mybir.AluOpType.mult, op1=mybir.AluOpType.add,
                )
                o_sb = opool.tile([P, OUT_CH], f32, tag="osb", name="o_sb")
                nc.vector.scalar_tensor_tensor(
                    o_sb[:], pacc[kj][:, 1:1 + OUT_CH], recip[:], t1[:],
                    op0=mybir.AluOpType.mult, op1=mybir.AluOpType.add,
                )
                nc.sync.dma_start(out_d[kj * P:(kj + 1) * P, :], o_sb[:])

    nc.compile()
    return nc


def _split2_f16(v):
    """2-way fp16 split: v ~= h1 + h2 with each half exactly fp16."""
    v = v.astype(np.float32)
    h1 = v.astype(np.float16)
    h2 = (v - h1.astype(np.float32)).astype(np.float16)
    return h1, h2


def _prepare_inputs(context_x, context_y, t, sigma, W, b):
    """Host prep: group channels by sigma, fold W, build per-core inputs."""
    sigma = np.asarray(sigma, dtype=np.float32)
    W64 = np.asarray(W, dtype=np.float64)
    b64 = np.asarray(b, dtype=np.float64)

    uniq = []
    for c in range(IN_CH):
        if sigma[c] not in uniq:
            uniq.append(sigma[c])
    uniq.sort(key=lambda s: (s != sigma[0]))  # channel-0 group first
    groups = [[c for c in range(IN_CH) if sigma[c] == s] for s in uniq]
    alphas = [0.5 / np.exp(2.0 * np.float64(s)) for s in uniq]
    widths = tuple((1 + OUT_CH) if 0 in g else OUT_CH for g in groups)
    G = len(groups)

    # fp16 path is safe unless sqrt(2a)*x or a*x^2 style terms overflow.
    xmax = max(
        float(np.abs(np.asarray(context_x)).max()),
        float(np.abs(np.asarray(t)).max()),
        1.0,
    )
    fp16_ok = all(a * xmax * xmax < 3e4 and np.isfinite(a) for a in alphas)

    in_maps = []
    for core in range(N_CORES):
        bidx, half = core // 2, core % 2
        x = np.asarray(context_x[bidx, :, 0], dtype=np.float64)
        th = np.asarray(t[bidx, half * O_CORE:(half + 1) * O_CORE, 0],
                        dtype=np.float64)
        y = np.asarray(context_y[bidx], dtype=np.float64)

        m = {}
        if fp16_ok:
            BLK = N_IN + O_CORE
            xr = np.empty((KEXP, G * BLK), dtype=np.float16)
            for g, a in enumerate(alphas):
                r = np.sqrt(2.0 * a)
                s1, s2 = _split2_f16(r * x)
                u1, u2 = _split2_f16(r * th)
                q1, q2 = _split2_f16(0.5 * (r * x) ** 2)
                w1, w2 = _split2_f16(0.5 * (r * th) ** 2)
                one_i = np.ones(N_IN, np.float16)
                neg1 = np.full(O_CORE, -1.0, np.float16)
                xr[:, g * BLK:g * BLK + N_IN] = np.stack(
                    [s1, s1, s2, s2, q1, q2, one_i, one_i]
                )
                xr[:, g * BLK + N_IN:(g + 1) * BLK] = np.stack(
                    [u1, u2, u1, u2, neg1, neg1, -w1, -w2]
                )
            m["xr"] = xr
        else:
            lx = np.stack([x, x * x, np.ones_like(x)]).astype(np.float32)
            rt = np.empty((3 * G, O_CORE), dtype=np.float32)
            for g, a in enumerate(alphas):
                rt[3 * g + 0] = 2.0 * a * th
                rt[3 * g + 1] = -a
                rt[3 * g + 2] = -a * th * th
            m["lx"], m["rt"] = lx, rt

        blocks = []
        for g, chans in enumerate(groups):
            w = widths[g]
            rhs = np.zeros((N_IN, w), dtype=np.float64)
            coff = 0
            if 0 in chans:
                rhs[:, 0] = 1.0
                coff = 1
            conv_ch = [c for c in chans if c > 0]
            if conv_ch:
                rhs[:, coff:] = y[:, [c - 1 for c in conv_ch]] @ W64[conv_ch, :]
            blocks.append(
                rhs.reshape(KI, P, w).transpose(1, 0, 2).reshape(P, KI * w)
            )
        ydt = np.concatenate(blocks, axis=1)
        m["ydt"] = ydt.astype(np.float16 if fp16_ok else np.float32)
        wb0 = np.tile(W64[0].astype(np.float32), (P, 1))
        bt = np.tile(b64.astype(np.float32), (P, 1))
        if fp16_ok:
            m["wbb"] = np.concatenate([wb0, bt], axis=1)
        else:
            m["wb0"], m["bt"] = wb0, bt
        in_maps.append(m)
    return widths, fp16_ok, in_maps


def _run(inputs: dict, trace: bool = False):
    """Compile (cached), run on 8 cores, gather. Returns (output, results)."""
    from concourse.bass_utils import run_bass_kernel_spmd

    widths, fp16_ok, in_maps = _prepare_inputs(
        inputs["context_x"], inputs["context_y"], inputs["t"],
        inputs["sigma"], inputs["W"], inputs["b"],
    )
    key = (widths, fp16_ok)
    if key not in _BASS_CACHE:
        _BASS_CACHE[key] = (_build_fp16_raw if fp16_ok else _build_fp32)(widths)
    nc = _BASS_CACHE[key]

    res = run_bass_kernel_spmd(nc, in_maps, list(range(N_CORES)), trace=trace)

    out = np.empty((B, N_OUT, OUT_CH), dtype=np.float32)
    for core in range(N_CORES):
        bidx, half = core // 2, core % 2
        out[bidx, half * O_CORE:(half + 1) * O_CORE, :] = res.results[core]["out"]
    return out, res


def kernel(**inputs) -> np.ndarray:
    out, _ = _run(inputs, trace=False)
    return out


# revision 17
# speedup vs baseline: 1.9960x; 1.0130x over previous
"""ConvDeepSet kernel for Trainium2 (8 NeuronCores, Bass/Tile).

Math (per batch b, target point o, channel c):
    agg[o,c] = sum_i yd[i,c] * exp(-alpha_c * (x_i - t_o)^2)      yd = [1 | y]
    out[o,:] = [agg0, agg1/(agg0+eps), ...] @ W + b

Channels sharing a sigma value share the RBF matrix E[i,o], so with G
sigma-groups the aggregation collapses to G matmuls after folding W into
the context values on the host:
    U_g = sum_{c in g, c>0} yd[:,c] W[c,:]              (n_in, 16)
    P[o, 0]    = den[o]  = sum_i E_g0[i,o]              (density group g0)
    P[o, 1+j] += sum_i E_g[i,o] U_g[i,j]                (all groups)
    out[o,:]  = den*W[0,:] + P[o,1:]/(den+eps) + b

The exponent -a(x-t)^2 = s*u - s^2/2 - u^2/2 with s = sqrt(2a)x, u =
sqrt(2a)t is a rank-8 fp16 matmul using 2-way fp16 splits of s, u and of
q = s^2/2, w = u^2/2 (host-prepared), exact to ~1e-4 absolute — fp32
quality at fp16 matmul speed (single pass + fast weight load). ScalarE
applies exp (PSUM -> SBUF, fp16 out); the aggregation runs in fp16 with
fp32 PSUM accumulation (max abs output error ~5e-4 on the reference data,
~5e-5 of the output scale).

Sharding: core c -> (batch c//2, output half c%2). Per-core data all
lives in SBUF; the (n_in, n_out, C) intermediate never materializes.
"""

import numpy as np

B, N_IN, N_OUT = 4, 1024, 1024
IN_CH_RAW, OUT_CH = 7, 16
IN_CH = IN_CH_RAW + 1
N_CORES = 8
O_CORE = N_OUT // 2          # 512 target points per core
P = 128                      # partitions
KI = N_IN // P               # 8 contraction chunks
NPAIR = KI // 2              # exp processed in chunk pairs
KJ = O_CORE // P             # 4 output chunks
KEXP = 8                     # rows of the split-product exponent matmul
EPS = 1e-8

_BASS_CACHE: dict = {}


def _build_fp16_raw(widths):
    """Raw-Bass (no Tile) version of the fp16 pipeline: hand-rolled
    semaphores, no Tile entry/exit barrier stages — saves ~10us of
    framework overhead at these kernel sizes."""
    import concourse.bass as bass
    from concourse import mybir

    f32 = mybir.dt.float32
    f16 = mybir.dt.float16
    G = len(widths)
    wtot = sum(widths)
    offs = np.cumsum([0] + list(widths))
    npair = NPAIR * G
    Exp = mybir.ActivationFunctionType.Exp

    nc = bass.Bass("TRN2", target_bir_lowering=False, debug=False)

    # xr = per-group [lxh | rth] blocks along the free dim (groups must sit at
    # partition 0 for PE); one DMA feeds the first matmul. wbb = [wb0 | bt].
    BLK = N_IN + O_CORE
    xr_d = nc.dram_tensor("xr", [KEXP, G * BLK], f16, kind="ExternalInput")
    ydt_d = nc.dram_tensor("ydt", [P, KI * wtot], f16, kind="ExternalInput")
    wbb_d = nc.dram_tensor("wbb", [P, 2 * OUT_CH], f32, kind="ExternalInput")
    out_d = nc.dram_tensor("out", [O_CORE, OUT_CH], f32, kind="ExternalOutput")

    xr = nc.alloc_sbuf_tensor("xr_sb", [KEXP, G * BLK], f16).ap()
    lxh = [xr[:, g * BLK:g * BLK + N_IN] for g in range(G)]
    rth = [xr[:, g * BLK + N_IN:(g + 1) * BLK] for g in range(G)]
    ydt = nc.alloc_sbuf_tensor("ydt_sb", [P, KI * wtot], f16).ap()
    wbb = nc.alloc_sbuf_tensor("wbb_sb", [P, 2 * OUT_CH], f32).ap()
    wb0 = wbb[:, :OUT_CH]
    bt = wbb[:, OUT_CH:]
    warm = nc.alloc_sbuf_tensor("warm_sb", [1, 1], f32).ap()
    E = [nc.alloc_sbuf_tensor(f"e_sb{q}", [P, 2 * O_CORE], f16).ap()
         for q in range(npair)]
    denp = [nc.alloc_sbuf_tensor(f"denp_sb{kj}", [P, 1], f32).ap()
            for kj in range(KJ)]
    recip = [nc.alloc_sbuf_tensor(f"recip_sb{kj}", [P, 1], f32).ap()
             for kj in range(KJ)]
    t1 = [nc.alloc_sbuf_tensor(f"t1_sb{kj}", [P, OUT_CH], f32).ap()
          for kj in range(KJ)]
    o_all = nc.alloc_sbuf_tensor("o_all_sb", [P, KJ * OUT_CH], f32).ap()
    o_sb = [o_all[:, kj * OUT_CH:(kj + 1) * OUT_CH] for kj in range(KJ)]
    A = [nc.alloc_psum_tensor(f"a_ps{i}", [P, 2 * O_CORE], f32).ap()
         for i in range(2)]
    pacc = [nc.alloc_psum_tensor(f"pacc{kj}", [P, 1 + OUT_CH], f32).ap()
            for kj in range(KJ)]

    with (
        nc.Block() as block,
        nc.semaphore("dsem_s") as dsem_s,   # sync-queue input DMAs
        nc.semaphore("dsem_g") as dsem_g,   # gpsimd-queue input DMAs
        nc.semaphore("psem") as psem,       # PE exponent matmuls done
        nc.semaphore("asem") as asem,       # ACT exp pairs done
        nc.semaphore("gsem") as gsem,       # PE agg per-kj done
        nc.semaphore("vsem") as vsem,       # DVE epilogue per-kj done
        nc.semaphore("vv") as vv,           # DVE same-engine pipeline sync
        nc.semaphore("osem") as osem,       # out DMAs done
    ):
        @block.sync
        def _(sync):
            sync.dma_start(out=xr[:], in_=xr_d[:]).then_inc(dsem_s, 16)
            # per-kj output DMAs fire as each epilogue chain lands, so the
            # kj0-2 stores overlap the remaining matmuls/epilogues
            for kj in range(KJ):
                sync.wait_ge(vsem, kj + 1)
                sync.dma_start(
                    out=out_d[kj * P:(kj + 1) * P, :], in_=o_sb[kj][:]
                ).then_inc(osem, 16)
            sync.wait_ge(osem, 16 * KJ)

        @block.gpsimd
        def _(gpsimd):
            gpsimd.dma_start(out=ydt[:], in_=ydt_d[:]).then_inc(dsem_g, 16)
            gpsimd.dma_start(out=wbb[:], in_=wbb_d[:]).then_inc(dsem_g, 16)

        @block.tensor
        def _(tensor):
            tensor.wait_ge(dsem_s, 16)
            for q in range(npair):
                g, p = divmod(q, NPAIR)
                if q >= 2:
                    tensor.wait_ge(asem, q - 1)  # A buffer q%2 free again
                for half in range(2):
                    ki = 2 * p + half
                    tensor.matmul(
                        A[q % 2][:, half * O_CORE:(half + 1) * O_CORE],
                        lxh[g][:, ki * P:(ki + 1) * P],
                        rth[g][:],
                        start=True,
                        stop=True,
                    ).then_inc(psem, 1)
            tensor.wait_ge(dsem_g, 32)  # ydt (all gpsimd-queue DMAs)
            n_mm = G * KI
            for kj in range(KJ):
                for g in range(G):
                    w = widths[g]
                    coff = 0 if g == 0 else 1
                    for ki in range(KI):
                        idx = g * KI + ki
                        q = g * NPAIR + ki // 2
                        if kj == 0 and ki % 2 == 0:
                            tensor.wait_ge(asem, q + 1)
                        rhs = ydt[:, KI * offs[g] + ki * w:
                                  KI * offs[g] + (ki + 1) * w]
                        lhs = E[q][:, (ki % 2) * O_CORE + kj * P:
                                   (ki % 2) * O_CORE + (kj + 1) * P]
                        mm = tensor.matmul(
                            pacc[kj][:, coff:coff + w],
                            lhs,
                            rhs,
                            start=(idx == 0),
                            stop=(idx == n_mm - 1),
                            skip_group_check=(G > 1),
                        )
                        if idx == n_mm - 1:
                            mm.then_inc(gsem, 1)

        @block.scalar
        def _(scalar):
            # touch Exp before the pipeline needs it: loads the ACT table
            # while the input DMAs are still in flight
            scalar.activation(warm[:], nc.const_aps.tensor(0.0, (1, 1)), Exp)
            for q in range(npair):
                scalar.wait_ge(psem, 2 * (q + 1))
                scalar.activation(E[q][:], A[q % 2][:], Exp).then_inc(asem, 1)

        @block.vector
        def _(vector):
            vector.wait_ge(dsem_g, 32)  # wbb resident
            # per-kj chain: runs as soon as that kj's PSUM bank is complete,
            # so kj0-2 finish during the remaining aggregation matmuls
            for kj in range(KJ):
                vector.wait_ge(gsem, kj + 1)
                vector.tensor_scalar_add(
                    denp[kj][:], pacc[kj][:, 0:1], EPS
                ).then_inc(vv, 1)
                vector.wait_ge(vv, 3 * kj + 1)  # denp through the pipe
                vector.scalar_tensor_tensor(
                    t1[kj][:], wb0[:], denp[kj][:], bt[:],
                    op0=mybir.AluOpType.mult, op1=mybir.AluOpType.add,
                ).then_inc(vv, 1)
                vector.reciprocal(recip[kj][:], denp[kj][:]).then_inc(vv, 1)
                vector.wait_ge(vv, 3 * kj + 3)  # t1 + recip through the pipe
                vector.scalar_tensor_tensor(
                    o_sb[kj][:], pacc[kj][:, 1:1 + OUT_CH], recip[kj][:], t1[kj][:],
                    op0=mybir.AluOpType.mult, op1=mybir.AluOpType.add,
                ).then_inc(vsem, 1)

    return nc


def _build_fp32(widths):
    """Fallback: fp32 rank-3 exponent matmul + fp32 aggregation (slower,
    used only when fp16 split values would overflow)."""
    import concourse.bacc as bacc
    import concourse.tile as tile
    from concourse import mybir

    f32 = mybir.dt.float32
    G = len(widths)
    wtot = sum(widths)
    offs = np.cumsum([0] + list(widths))

    nc = bacc.Bacc("TRN2", target_bir_lowering=False, debug=False)

    lx_d = nc.dram_tensor("lx", [3, N_IN], f32, kind="ExternalInput")
    rt_d = nc.dram_tensor("rt", [3 * G, O_CORE], f32, kind="ExternalInput")
    ydt_d = nc.dram_tensor("ydt", [P, KI * wtot], f32, kind="ExternalInput")
    wb0_d = nc.dram_tensor("wb0", [P, OUT_CH], f32, kind="ExternalInput")
    bt_d = nc.dram_tensor("bt", [P, OUT_CH], f32, kind="ExternalInput")
    out_d = nc.dram_tensor("out", [O_CORE, OUT_CH], f32, kind="ExternalOutput")

    with tile.TileContext(nc) as tc:
        with (
            tc.tile_pool(name="const", bufs=1) as cpool,
            tc.tile_pool(name="epool", bufs=1) as epool,
            tc.tile_pool(name="small", bufs=2) as spool,
            tc.tile_pool(name="outp", bufs=2) as opool,
            tc.tile_pool(name="apsum", bufs=3, space="PSUM") as apsum,
            tc.tile_pool(name="ppsum", bufs=1, space="PSUM") as ppsum,
        ):
            lx = cpool.tile([3, N_IN], f32, tag="lx")
            nc.sync.dma_start(lx[:], lx_d[:])
            rt = cpool.tile([3 * G, O_CORE], f32, tag="rt")
            nc.scalar.dma_start(rt[:], rt_d[:])
            ydt = cpool.tile([P, KI * wtot], f32, tag="ydt")
            nc.gpsimd.dma_start(ydt[:], ydt_d[:])
            wb0 = cpool.tile([P, OUT_CH], f32, tag="wb0")
            nc.gpsimd.dma_start(wb0[:], wb0_d[:])
            bt = cpool.tile([P, OUT_CH], f32, tag="bt")
            nc.gpsimd.dma_start(bt[:], bt_d[:])

            E = {}
            for g in range(G):
                for ki in range(KI):
                    a_ps = apsum.tile([P, O_CORE], f32, tag="A", name="a_ps")
                    nc.tensor.matmul(
                        a_ps[:],
                        lx[:, ki * P:(ki + 1) * P],
                        rt[3 * g:3 * g + 3, :],
                        start=True,
                        stop=True,
                    )
                    e = epool.tile([P, O_CORE], f32, tag=f"E{g}_{ki}", name="e")
                    nc.scalar.activation(
                        e[:], a_ps[:], mybir.ActivationFunctionType.Exp
                    )
                    E[(g, ki)] = e

            pacc = [
                ppsum.tile([P, 1 + OUT_CH], f32, tag=f"P{kj}", name=f"pacc{kj}")
                for kj in range(KJ)
            ]
            n_mm = G * KI
            for kj in range(KJ):
                for g in range(G):
                    w = widths[g]
                    coff = 0 if g == 0 else 1
                    for ki in range(KI):
                        idx = g * KI + ki
                        rhs = ydt[:, KI * offs[g] + ki * w: KI * offs[g] + (ki + 1) * w]
                        nc.tensor.matmul(
                            pacc[kj][:, coff:coff + w],
                            E[(g, ki)][:, kj * P:(kj + 1) * P],
                            rhs,
                            start=(idx == 0),
                            stop=(idx == n_mm - 1),
                            skip_group_check=(G > 1),
                        )

                denp = spool.tile([P, 1], f32, tag="denp", name="denp")
                nc.vector.tensor_scalar_add(denp[:], pacc[kj][:, 0:1], EPS)
                recip = spool.tile([P, 1], f32, tag="recip", name="recip")
                nc.vector.reciprocal(recip[:], denp[:])
                t1 = spool.tile([P, OUT_CH], f32, tag="t1", name="t1")
                nc.vector.scalar_tensor_tensor(
                    t1[:], wb0[:], denp[:], bt[:],
                    op0=mybir.AluOpType.mult, op1=mybir.AluOpType.add,
                )
                o_sb = opool.tile([P, OUT_CH], f32, tag="osb", name="o_sb")
                nc.vector.scalar_tensor_tensor(
                    o_sb[:], pacc[kj][:, 1:1 + OUT_CH], recip[:], t1[:],
                    op0=mybir.AluOpType.mult, op1=mybir.AluOpType.add,
                )
                nc.sync.dma_start(out_d[kj * P:(kj + 1) * P, :], o_sb[:])

    nc.compile()
    return nc


def _split2_f16(v):
    """2-way fp16 split: v ~= h1 + h2 with each half exactly fp16."""
    v = v.astype(np.float32)
    h1 = v.astype(np.float16)
    h2 = (v - h1.astype(np.float32)).astype(np.float16)
    return h1, h2


def _prepare_inputs(context_x, context_y, t, sigma, W, b):
    """Host prep: group channels by sigma, fold W, build per-core inputs."""
    sigma = np.asarray(sigma, dtype=np.float32)
    W64 = np.asarray(W, dtype=np.float64)
    b64 = np.asarray(b, dtype=np.float64)

    uniq = []
    for c in range(IN_CH):
        if sigma[c] not in uniq:
            uniq.append(sigma[c])
    uniq.sort(key=lambda s: (s != sigma[0]))  # channel-0 group first
    groups = [[c for c in range(IN_CH) if sigma[c] == s] for s in uniq]
    alphas = [0.5 / np.exp(2.0 * np.float64(s)) for s in uniq]
    widths = tuple((1 + OUT_CH) if 0 in g else OUT_CH for g in groups)
    G = len(groups)

    # fp16 path is safe unless sqrt(2a)*x or a*x^2 style terms overflow.
    xmax = max(
        float(np.abs(np.asarray(context_x)).max()),
        float(np.abs(np.asarray(t)).max()),
        1.0,
    )
    fp16_ok = all(a * xmax * xmax < 3e4 and np.isfinite(a) for a in alphas)

    in_maps = []
    for core in range(N_CORES):
        bidx, half = core // 2, core % 2
        x = np.asarray(context_x[bidx, :, 0], dtype=np.float64)
        th = np.asarray(t[bidx, half * O_CORE:(half + 1) * O_CORE, 0],
                        dtype=np.float64)
        y = np.asarray(context_y[bidx], dtype=np.float64)

        m = {}
        if fp16_ok:
            BLK = N_IN + O_CORE
            xr = np.empty((KEXP, G * BLK), dtype=np.float16)
            for g, a in enumerate(alphas):
                r = np.sqrt(2.0 * a)
                s1, s2 = _split2_f16(r * x)
                u1, u2 = _split2_f16(r * th)
                q1, q2 = _split2_f16(0.5 * (r * x) ** 2)
                w1, w2 = _split2_f16(0.5 * (r * th) ** 2)
                one_i = np.ones(N_IN, np.float16)
                neg1 = np.full(O_CORE, -1.0, np.float16)
                xr[:, g * BLK:g * BLK + N_IN] = np.stack(
                    [s1, s1, s2, s2, q1, q2, one_i, one_i]
                )
                xr[:, g * BLK + N_IN:(g + 1) * BLK] = np.stack(
                    [u1, u2, u1, u2, neg1, neg1, -w1, -w2]
                )
            m["xr"] = xr
        else:
            lx = np.stack([x, x * x, np.ones_like(x)]).astype(np.float32)
            rt = np.empty((3 * G, O_CORE), dtype=np.float32)
            for g, a in enumerate(alphas):
                rt[3 * g + 0] = 2.0 * a * th
                rt[3 * g + 1] = -a
                rt[3 * g + 2] = -a * th * th
            m["lx"], m["rt"] = lx, rt

        blocks = []
        for g, chans in enumerate(groups):
            w = widths[g]
            rhs = np.zeros((N_IN, w), dtype=np.float64)
            coff = 0
            if 0 in chans:
                rhs[:, 0] = 1.0
                coff = 1
            conv_ch = [c for c in chans if c > 0]
            if conv_ch:
                rhs[:, coff:] = y[:, [c - 1 for c in conv_ch]] @ W64[conv_ch, :]
            blocks.append(
                rhs.reshape(KI, P, w).transpose(1, 0, 2).reshape(P, KI * w)
            )
        ydt = np.concatenate(blocks, axis=1)
        m["ydt"] = ydt.astype(np.float16 if fp16_ok else np.float32)
        wb0 = np.tile(W64[0].astype(np.float32), (P, 1))
        bt = np.tile(b64.astype(np.float32), (P, 1))
        if fp16_ok:
            m["wbb"] = np.concatenate([wb0, bt], axis=1)
        else:
            m["wb0"], m["bt"] = wb0, bt
        in_maps.append(m)
    return widths, fp16_ok, in_maps


def _run(inputs: dict, trace: bool = False):
    """Compile (cached), run on 8 cores, gather. Returns (output, results)."""
    from concourse.bass_utils import run_bass_kernel_spmd

    widths, fp16_ok, in_maps = _prepare_inputs(
        inputs["context_x"], inputs["context_y"], inputs["t"],
        inputs["sigma"], inputs["W"], inputs["b"],
    )
    key = (widths, fp16_ok)
    if key not in _BASS_CACHE:
        _BASS_CACHE[key] = (_build_fp16_raw if fp16_ok else _build_fp32)(widths)
    nc = _BASS_CACHE[key]

    res = run_bass_kernel_spmd(nc, in_maps, list(range(N_CORES)), trace=trace)

    out = np.empty((B, N_OUT, OUT_CH), dtype=np.float32)
    for core in range(N_CORES):
        bidx, half = core // 2, core % 2
        out[bidx, half * O_CORE:(half + 1) * O_CORE, :] = res.results[core]["out"]
    return out, res


def kernel(**inputs) -> np.ndarray:
    out, _ = _run(inputs, trace=False)
    return out


# revision 18
# speedup vs baseline: 2.0568x; 1.0305x over previous
"""ConvDeepSet kernel for Trainium2 (8 NeuronCores, Bass/Tile).

Math (per batch b, target point o, channel c):
    agg[o,c] = sum_i yd[i,c] * exp(-alpha_c * (x_i - t_o)^2)      yd = [1 | y]
    out[o,:] = [agg0, agg1/(agg0+eps), ...] @ W + b

Channels sharing a sigma value share the RBF matrix E[i,o], so with G
sigma-groups the aggregation collapses to G matmuls after folding W into
the context values on the host:
    U_g = sum_{c in g, c>0} yd[:,c] W[c,:]              (n_in, 16)
    P[o, 0]    = den[o]  = sum_i E_g0[i,o]              (density group g0)
    P[o, 1+j] += sum_i E_g[i,o] U_g[i,j]                (all groups)
    out[o,:]  = den*W[0,:] + P[o,1:]/(den+eps) + b

The exponent -a(x-t)^2 = s*u - s^2/2 - u^2/2 with s = sqrt(2a)x, u =
sqrt(2a)t is a rank-8 fp16 matmul using 2-way fp16 splits of s, u and of
q = s^2/2, w = u^2/2 (host-prepared), exact to ~1e-4 absolute — fp32
quality at fp16 matmul speed (single pass + fast weight load). ScalarE
applies exp (PSUM -> SBUF, fp16 out); the aggregation runs in fp16 with
fp32 PSUM accumulation (max abs output error ~5e-4 on the reference data,
~5e-5 of the output scale).

Sharding: core c -> (batch c//2, output half c%2). Per-core data all
lives in SBUF; the (n_in, n_out, C) intermediate never materializes.
"""

import numpy as np

B, N_IN, N_OUT = 4, 1024, 1024
IN_CH_RAW, OUT_CH = 7, 16
IN_CH = IN_CH_RAW + 1
N_CORES = 8
O_CORE = N_OUT // 2          # 512 target points per core
P = 128                      # partitions
KI = N_IN // P               # 8 contraction chunks
NPAIR = KI // 2              # exp processed in chunk pairs
KJ = O_CORE // P             # 4 output chunks
KEXP = 8                     # rows of the split-product exponent matmul
EPS = 1e-8

_BASS_CACHE: dict = {}


def _build_fp16_raw(widths):
    """Raw-Bass (no Tile) version of the fp16 pipeline: hand-rolled
    semaphores, no Tile entry/exit barrier stages — saves ~10us of
    framework overhead at these kernel sizes."""
    import concourse.bass as bass
    from concourse import mybir

    f32 = mybir.dt.float32
    f16 = mybir.dt.float16
    G = len(widths)
    wtot = sum(widths)
    offs = np.cumsum([0] + list(widths))
    npair = NPAIR * G
    Exp = mybir.ActivationFunctionType.Exp

    nc = bass.Bass("TRN2", target_bir_lowering=False, debug=False)

    # xr = per-group [lxh | rth] blocks along the free dim (groups must sit at
    # partition 0 for PE); one DMA feeds the first matmul. wbb = [wb0 | bt].
    BLK = N_IN + O_CORE
    xr_d = nc.dram_tensor("xr", [KEXP, G * BLK], f16, kind="ExternalInput")
    ydt_d = nc.dram_tensor("ydt", [P, KI * wtot], f16, kind="ExternalInput")
    wbb_d = nc.dram_tensor("wbb", [P, 2 * OUT_CH], f32, kind="ExternalInput")
    out_d = nc.dram_tensor("out", [O_CORE, OUT_CH], f32, kind="ExternalOutput")

    xr = nc.alloc_sbuf_tensor("xr_sb", [KEXP, G * BLK], f16).ap()
    lxh = [xr[:, g * BLK:g * BLK + N_IN] for g in range(G)]
    rth = [xr[:, g * BLK + N_IN:(g + 1) * BLK] for g in range(G)]
    ydt = nc.alloc_sbuf_tensor("ydt_sb", [P, KI * wtot], f16).ap()
    wbb = nc.alloc_sbuf_tensor("wbb_sb", [P, 2 * OUT_CH], f32).ap()
    wb0 = wbb[:, :OUT_CH]
    bt = wbb[:, OUT_CH:]
    warm = nc.alloc_sbuf_tensor("warm_sb", [1, 1], f32).ap()
    E = [nc.alloc_sbuf_tensor(f"e_sb{q}", [P, 2 * O_CORE], f16).ap()
         for q in range(npair)]
    denp = [nc.alloc_sbuf_tensor(f"denp_sb{kj}", [P, 1], f32).ap()
            for kj in range(KJ)]
    recip = [nc.alloc_sbuf_tensor(f"recip_sb{kj}", [P, 1], f32).ap()
             for kj in range(KJ)]
    t1 = [nc.alloc_sbuf_tensor(f"t1_sb{kj}", [P, OUT_CH], f32).ap()
          for kj in range(KJ)]
    o_all = nc.alloc_sbuf_tensor("o_all_sb", [P, KJ * OUT_CH], f32).ap()
    o_sb = [o_all[:, kj * OUT_CH:(kj + 1) * OUT_CH] for kj in range(KJ)]
    A = [nc.alloc_psum_tensor(f"a_ps{i}", [P, 2 * O_CORE], f32).ap()
         for i in range(2)]
    pacc = [nc.alloc_psum_tensor(f"pacc{kj}", [P, 1 + OUT_CH], f32).ap()
            for kj in range(KJ)]

    with (
        nc.Block() as block,
        nc.semaphore("dsem_s") as dsem_s,   # sync-queue input DMAs
        nc.semaphore("dsem_g") as dsem_g,   # gpsimd-queue input DMAs
        nc.semaphore("psem") as psem,       # PE exponent matmuls done
        nc.semaphore("asem") as asem,       # ACT exp pairs done
        nc.semaphore("gsem") as gsem,       # PE agg per-kj done
        nc.semaphore("vsem") as vsem,       # DVE epilogue per-kj done
        nc.semaphore("vv") as vv,           # DVE same-engine pipeline sync
        nc.semaphore("osem") as osem,       # out DMAs done
    ):
        @block.sync
        def _(sync):
            sync.dma_start(out=xr[:], in_=xr_d[:]).then_inc(dsem_s, 16)
            # per-kj output DMAs fire as each epilogue chain lands, so the
            # kj0-2 stores overlap the remaining matmuls/epilogues
            for kj in range(KJ):
                sync.wait_ge(vsem, kj + 1)
                sync.dma_start(
                    out=out_d[kj * P:(kj + 1) * P, :], in_=o_sb[kj][:]
                ).then_inc(osem, 16)
            # no final osem wait: the Block-exit DRAIN on SP already blocks
            # until the DGE queues are empty, and the semaphore completion
            # path adds ~1.8us of latency on top of the actual transfer

        @block.gpsimd
        def _(gpsimd):
            gpsimd.dma_start(out=ydt[:], in_=ydt_d[:]).then_inc(dsem_g, 16)
            gpsimd.dma_start(out=wbb[:], in_=wbb_d[:]).then_inc(dsem_g, 16)

        @block.tensor
        def _(tensor):
            tensor.wait_ge(dsem_s, 16)
            for q in range(npair):
                g, p = divmod(q, NPAIR)
                if q >= 2:
                    tensor.wait_ge(asem, q - 1)  # A buffer q%2 free again
                for half in range(2):
                    ki = 2 * p + half
                    tensor.matmul(
                        A[q % 2][:, half * O_CORE:(half + 1) * O_CORE],
                        lxh[g][:, ki * P:(ki + 1) * P],
                        rth[g][:],
                        start=True,
                        stop=True,
                    ).then_inc(psem, 1)
            tensor.wait_ge(dsem_g, 32)  # ydt (all gpsimd-queue DMAs)
            n_mm = G * KI
            for kj in range(KJ):
                for g in range(G):
                    w = widths[g]
                    coff = 0 if g == 0 else 1
                    for ki in range(KI):
                        idx = g * KI + ki
                        q = g * NPAIR + ki // 2
                        if kj == 0 and ki % 2 == 0:
                            tensor.wait_ge(asem, q + 1)
                        rhs = ydt[:, KI * offs[g] + ki * w:
                                  KI * offs[g] + (ki + 1) * w]
                        lhs = E[q][:, (ki % 2) * O_CORE + kj * P:
                                   (ki % 2) * O_CORE + (kj + 1) * P]
                        mm = tensor.matmul(
                            pacc[kj][:, coff:coff + w],
                            lhs,
                            rhs,
                            start=(idx == 0),
                            stop=(idx == n_mm - 1),
                            skip_group_check=(G > 1),
                        )
                        if idx == n_mm - 1:
                            mm.then_inc(gsem, 1)

        @block.scalar
        def _(scalar):
            # touch Exp before the pipeline needs it: loads the ACT table
            # while the input DMAs are still in flight
            scalar.activation(warm[:], nc.const_aps.tensor(0.0, (1, 1)), Exp)
            for q in range(npair):
                scalar.wait_ge(psem, 2 * (q + 1))
                scalar.activation(E[q][:], A[q % 2][:], Exp).then_inc(asem, 1)

        @block.vector
        def _(vector):
            vector.wait_ge(dsem_g, 32)  # wbb resident
            # per-kj chain: runs as soon as that kj's PSUM bank is complete,
            # so kj0-2 finish during the remaining aggregation matmuls
            for kj in range(KJ):
                vector.wait_ge(gsem, kj + 1)
                vector.tensor_scalar_add(
                    denp[kj][:], pacc[kj][:, 0:1], EPS
                ).then_inc(vv, 1)
                vector.wait_ge(vv, 3 * kj + 1)  # denp through the pipe
                vector.scalar_tensor_tensor(
                    t1[kj][:], wb0[:], denp[kj][:], bt[:],
                    op0=mybir.AluOpType.mult, op1=mybir.AluOpType.add,
                ).then_inc(vv, 1)
                vector.reciprocal(recip[kj][:], denp[kj][:]).then_inc(vv, 1)
                vector.wait_ge(vv, 3 * kj + 3)  # t1 + recip through the pipe
                vector.scalar_tensor_tensor(
                    o_sb[kj][:], pacc[kj][:, 1:1 + OUT_CH], recip[kj][:], t1[kj][:],
                    op0=mybir.AluOpType.mult, op1=mybir.AluOpType.add,
                ).then_inc(vsem, 1)

    return nc


def _build_fp32(widths):
    """Fallback: fp32 rank-3 exponent matmul + fp32 aggregation (slower,
    used only when fp16 split values would overflow)."""
    import concourse.bacc as bacc
    import concourse.tile as tile
    from concourse import mybir

    f32 = mybir.dt.float32
    G = len(widths)
    wtot = sum(widths)
    offs = np.cumsum([0] + list(widths))

    nc = bacc.Bacc("TRN2", target_bir_lowering=False, debug=False)

    lx_d = nc.dram_tensor("lx", [3, N_IN], f32, kind="ExternalInput")
    rt_d = nc.dram_tensor("rt", [3 * G, O_CORE], f32, kind="ExternalInput")
    ydt_d = nc.dram_tensor("ydt", [P, KI * wtot], f32, kind="ExternalInput")
    wb0_d = nc.dram_tensor("wb0", [P, OUT_CH], f32, kind="ExternalInput")
    bt_d = nc.dram_tensor("bt", [P, OUT_CH], f32, kind="ExternalInput")
    out_d = nc.dram_tensor("out", [O_CORE, OUT_CH], f32, kind="ExternalOutput")

    with tile.TileContext(nc) as tc:
        with (
            tc.tile_pool(name="const", bufs=1) as cpool,
            tc.tile_pool(name="epool", bufs=1) as epool,
            tc.tile_pool(name="small", bufs=2) as spool,
            tc.tile_pool(name="outp", bufs=2) as opool,
            tc.tile_pool(name="apsum", bufs=3, space="PSUM") as apsum,
            tc.tile_pool(name="ppsum", bufs=1, space="PSUM") as ppsum,
        ):
            lx = cpool.tile([3, N_IN], f32, tag="lx")
            nc.sync.dma_start(lx[:], lx_d[:])
            rt = cpool.tile([3 * G, O_CORE], f32, tag="rt")
            nc.scalar.dma_start(rt[:], rt_d[:])
            ydt = cpool.tile([P, KI * wtot], f32, tag="ydt")
            nc.gpsimd.dma_start(ydt[:], ydt_d[:])
            wb0 = cpool.tile([P, OUT_CH], f32, tag="wb0")
            nc.gpsimd.dma_start(wb0[:], wb0_d[:])
            bt = cpool.tile([P, OUT_CH], f32, tag="bt")
            nc.gpsimd.dma_start(bt[:], bt_d[:])

            E = {}
            for g in range(G):
                for ki in range(KI):
                    a_ps = apsum.tile([P, O_CORE], f32, tag="A", name="a_ps")
                    nc.tensor.matmul(
                        a_ps[:],
                        lx[:, ki * P:(ki + 1) * P],
                        rt[3 * g:3 * g + 3, :],
                        start=True,
                        stop=True,
                    )
                    e = epool.tile([P, O_CORE], f32, tag=f"E{g}_{ki}", name="e")
                    nc.scalar.activation(
                        e[:], a_ps[:], mybir.ActivationFunctionType.Exp
                    )
                    E[(g, ki)] = e

            pacc = [
                ppsum.tile([P, 1 + OUT_CH], f32, tag=f"P{kj}", name=f"pacc{kj}")
                for kj in range(KJ)
            ]
            n_mm = G * KI
            for kj in range(KJ):
                for g in range(G):
                    w = widths[g]
                    coff = 0 if g == 0 else 1
                    for ki in range(KI):
                        idx = g * KI + ki
                        rhs = ydt[:, KI * offs[g] + ki * w: KI * offs[g] + (ki + 1) * w]
                        nc.tensor.matmul(
                            pacc[kj][:, coff:coff + w],
                            E[(g, ki)][:, kj * P:(kj + 1) * P],
                            rhs,
                            start=(idx == 0),
                            stop=(idx == n_mm - 1),
                            skip_group_check=(G > 1),
                        )

                denp = spool.tile([P, 1], f32, tag="denp", name="denp")
                nc.vector.tensor_scalar_add(denp[:], pacc[kj][:, 0:1], EPS)
                recip = spool.tile([P, 1], f32, tag="recip", name="recip")
                nc.vector.reciprocal(recip[:], denp[:])
                t1 = spool.tile([P, OUT_CH], f32, tag="t1", name="t1")
                nc.vector.scalar_tensor_tensor(
                    t1[:], wb0[:], denp[:], bt[:],
                    op0=mybir.AluOpType.mult, op1=mybir.AluOpType.add,
                )
                o_sb = opool.tile([P, OUT_CH], f32, tag="osb", name="o_sb")
                nc.vector.scalar_tensor_tensor(
                    o_sb[:], pacc[kj][:, 1:1 + OUT_CH], recip[:], t1[:],
                    op0=mybir.AluOpType.mult, op1=mybir.AluOpType.add,
                )
                nc.sync.dma_start(out_d[kj * P:(kj + 1) * P, :], o_sb[:])

    nc.compile()
    return nc


def _split2_f16(v):
    """2-way fp16 split: v ~= h1 + h2 with each half exactly fp16."""
    v = v.astype(np.float32)
    h1 = v.astype(np.float16)
    h2 = (v - h1.astype(np.float32)).astype(np.float16)
    return h1, h2


def _prepare_inputs(context_x, context_y, t, sigma, W, b):
    """Host prep: group channels by sigma, fold W, build per-core inputs."""
    sigma = np.asarray(sigma, dtype=np.float32)
    W64 = np.asarray(W, dtype=np.float64)
    b64 = np.asarray(b, dtype=np.float64)

    uniq = []
    for c in range(IN_CH):
        if sigma[c] not in uniq:
            uniq.append(sigma[c])
    uniq.sort(key=lambda s: (s != sigma[0]))  # channel-0 group first
    groups = [[c for c in range(IN_CH) if sigma[c] == s] for s in uniq]
    alphas = [0.5 / np.exp(2.0 * np.float64(s)) for s in uniq]
    widths = tuple((1 + OUT_CH) if 0 in g else OUT_CH for g in groups)
    G = len(groups)

    # fp16 path is safe unless sqrt(2a)*x or a*x^2 style terms overflow.
    xmax = max(
        float(np.abs(np.asarray(context_x)).max()),
        float(np.abs(np.asarray(t)).max()),
        1.0,
    )
    fp16_ok = all(a * xmax * xmax < 3e4 and np.isfinite(a) for a in alphas)

    in_maps = []
    for core in range(N_CORES):
        bidx, half = core // 2, core % 2
        x = np.asarray(context_x[bidx, :, 0], dtype=np.float64)
        th = np.asarray(t[bidx, half * O_CORE:(half + 1) * O_CORE, 0],
                        dtype=np.float64)
        y = np.asarray(context_y[bidx], dtype=np.float64)

        m = {}
        if fp16_ok:
            BLK = N_IN + O_CORE
            xr = np.empty((KEXP, G * BLK), dtype=np.float16)
            for g, a in enumerate(alphas):
                r = np.sqrt(2.0 * a)
                s1, s2 = _split2_f16(r * x)
                u1, u2 = _split2_f16(r * th)
                q1, q2 = _split2_f16(0.5 * (r * x) ** 2)
                w1, w2 = _split2_f16(0.5 * (r * th) ** 2)
                one_i = np.ones(N_IN, np.float16)
                neg1 = np.full(O_CORE, -1.0, np.float16)
                xr[:, g * BLK:g * BLK + N_IN] = np.stack(
                    [s1, s1, s2, s2, q1, q2, one_i, one_i]
                )
                xr[:, g * BLK + N_IN:(g + 1) * BLK] = np.stack(
                    [u1, u2, u1, u2, neg1, neg1, -w1, -w2]
                )
            m["xr"] = xr
        else:
            lx = np.stack([x, x * x, np.ones_like(x)]).astype(np.float32)
            rt = np.empty((3 * G, O_CORE), dtype=np.float32)
            for g, a in enumerate(alphas):
                rt[3 * g + 0] = 2.0 * a * th
                rt[3 * g + 1] = -a
                rt[3 * g + 2] = -a * th * th
            m["lx"], m["rt"] = lx, rt

        blocks = []
        for g, chans in enumerate(groups):
            w = widths[g]
            rhs = np.zeros((N_IN, w), dtype=np.float64)
            coff = 0
            if 0 in chans:
                rhs[:, 0] = 1.0
                coff = 1
            conv_ch = [c for c in chans if c > 0]
            if conv_ch:
                rhs[:, coff:] = y[:, [c - 1 for c in conv_ch]] @ W64[conv_ch, :]
            blocks.append(
                rhs.reshape(KI, P, w).transpose(1, 0, 2).reshape(P, KI * w)
            )
        ydt = np.concatenate(blocks, axis=1)
        m["ydt"] = ydt.astype(np.float16 if fp16_ok else np.float32)
        wb0 = np.tile(W64[0].astype(np.float32), (P, 1))
        bt = np.tile(b64.astype(np.float32), (P, 1))
        if fp16_ok:
            m["wbb"] = np.concatenate([wb0, bt], axis=1)
        else:
            m["wb0"], m["bt"] = wb0, bt
        in_maps.append(m)
    return widths, fp16_ok, in_maps


def _run(inputs: dict, trace: bool = False):
    """Compile (cached), run on 8 cores, gather. Returns (output, results)."""
    from concourse.bass_utils import run_bass_kernel_spmd

    widths, fp16_ok, in_maps = _prepare_inputs(
        inputs["context_x"], inputs["context_y"], inputs["t"],
        inputs["sigma"], inputs["W"], inputs["b"],
    )
    key = (widths, fp16_ok)
    if key not in _BASS_CACHE:
        _BASS_CACHE[key] = (_build_fp16_raw if fp16_ok else _build_fp32)(widths)
    nc = _BASS_CACHE[key]

    res = run_bass_kernel_spmd(nc, in_maps, list(range(N_CORES)), trace=trace)

    out = np.empty((B, N_OUT, OUT_CH), dtype=np.float32)
    for core in range(N_CORES):
        bidx, half = core // 2, core % 2
        out[bidx, half * O_CORE:(half + 1) * O_CORE, :] = res.results[core]["out"]
    return out, res


def kernel(**inputs) -> np.ndarray:
    out, _ = _run(inputs, trace=False)
    return out


# revision 21
# speedup vs baseline: 2.0597x; 1.0014x over previous
"""ConvDeepSet kernel for Trainium2 (8 NeuronCores, Bass/Tile).

Math (per batch b, target point o, channel c):
    agg[o,c] = sum_i yd[i,c] * exp(-alpha_c * (x_i - t_o)^2)      yd = [1 | y]
    out[o,:] = [agg0, agg1/(agg0+eps), ...] @ W + b

Channels sharing a sigma value share the RBF matrix E[i,o], so with G
sigma-groups the aggregation collapses to G matmuls after folding W into
the context values on the host:
    U_g = sum_{c in g, c>0} yd[:,c] W[c,:]              (n_in, 16)
    P[o, 0]    = den[o]  = sum_i E_g0[i,o]              (density group g0)
    P[o, 1+j] += sum_i E_g[i,o] U_g[i,j]                (all groups)
    out[o,:]  = den*W[0,:] + P[o,1:]/(den+eps) + b

The exponent -a(x-t)^2 = s*u - s^2/2 - u^2/2 with s = sqrt(2a)x, u =
sqrt(2a)t is a rank-8 fp16 matmul using 2-way fp16 splits of s, u and of
q = s^2/2, w = u^2/2 (host-prepared), exact to ~1e-4 absolute — fp32
quality at fp16 matmul speed (single pass + fast weight load). ScalarE
applies exp (PSUM -> SBUF, fp16 out); the aggregation runs in fp16 with
fp32 PSUM accumulation (max abs output error ~5e-4 on the reference data,
~5e-5 of the output scale).

Sharding: core c -> (batch c//2, output half c%2). Per-core data all
lives in SBUF; the (n_in, n_out, C) intermediate never materializes.
"""

import numpy as np

B, N_IN, N_OUT = 4, 1024, 1024
IN_CH_RAW, OUT_CH = 7, 16
IN_CH = IN_CH_RAW + 1
N_CORES = 8
O_CORE = N_OUT // 2          # 512 target points per core
P = 128                      # partitions
KI = N_IN // P               # 8 contraction chunks
NPAIR = KI // 2              # exp processed in chunk pairs
KJ = O_CORE // P             # 4 output chunks
KEXP = 8                     # rows of the split-product exponent matmul
EPS = 1e-8

_BASS_CACHE: dict = {}


def _build_fp16_raw(widths):
    """Raw-Bass (no Tile) version of the fp16 pipeline: hand-rolled
    semaphores, no Tile entry/exit barrier stages — saves ~10us of
    framework overhead at these kernel sizes."""
    import concourse.bass as bass
    from concourse import mybir

    f32 = mybir.dt.float32
    f16 = mybir.dt.float16
    G = len(widths)
    wtot = sum(widths)
    offs = np.cumsum([0] + list(widths))
    npair = NPAIR * G
    Exp = mybir.ActivationFunctionType.Exp

    nc = bass.Bass("TRN2", target_bir_lowering=False, debug=False)

    # xr = per-group [lxh | rth] blocks along the free dim (groups must sit at
    # partition 0 for PE); one DMA feeds the first matmul. wbb = [wb0 | bt].
    BLK = N_IN + O_CORE
    xr_d = nc.dram_tensor("xr", [KEXP, G * BLK], f16, kind="ExternalInput")
    ydt_d = nc.dram_tensor("ydt", [P, KI * wtot], f16, kind="ExternalInput")
    wbb_d = nc.dram_tensor("wbb", [P, 2 * OUT_CH], f32, kind="ExternalInput")
    out_d = nc.dram_tensor("out", [O_CORE, OUT_CH], f32, kind="ExternalOutput")

    xr = nc.alloc_sbuf_tensor("xr_sb", [KEXP, G * BLK], f16).ap()
    lxh = [xr[:, g * BLK:g * BLK + N_IN] for g in range(G)]
    rth = [xr[:, g * BLK + N_IN:(g + 1) * BLK] for g in range(G)]
    ydt = nc.alloc_sbuf_tensor("ydt_sb", [P, KI * wtot], f16).ap()
    wbb = nc.alloc_sbuf_tensor("wbb_sb", [P, 2 * OUT_CH], f32).ap()
    wb0 = wbb[:, :OUT_CH]
    bt = wbb[:, OUT_CH:]
    warm = nc.alloc_sbuf_tensor("warm_sb", [1, 1], f32).ap()
    E = [nc.alloc_sbuf_tensor(f"e_sb{q}", [P, 2 * O_CORE], f16).ap()
         for q in range(npair)]
    denp = [nc.alloc_sbuf_tensor(f"denp_sb{kj}", [P, 1], f32).ap()
            for kj in range(KJ)]
    recip = [nc.alloc_sbuf_tensor(f"recip_sb{kj}", [P, 1], f32).ap()
             for kj in range(KJ)]
    t1 = [nc.alloc_sbuf_tensor(f"t1_sb{kj}", [P, OUT_CH], f32).ap()
          for kj in range(KJ)]
    o_all = nc.alloc_sbuf_tensor("o_all_sb", [P, KJ * OUT_CH], f32).ap()
    o_sb = [o_all[:, kj * OUT_CH:(kj + 1) * OUT_CH] for kj in range(KJ)]
    A = [nc.alloc_psum_tensor(f"a_ps{i}", [P, 2 * O_CORE], f32).ap()
         for i in range(2)]
    pacc = [nc.alloc_psum_tensor(f"pacc{kj}", [P, 1 + OUT_CH], f32).ap()
            for kj in range(KJ)]

    with (
        nc.Block() as block,
        nc.semaphore("dsem_s") as dsem_s,   # sync-queue input DMAs
        nc.semaphore("dsem_g") as dsem_g,   # gpsimd-queue input DMAs
        nc.semaphore("psem") as psem,       # PE exponent matmuls done
        nc.semaphore("asem") as asem,       # ACT exp pairs done
        nc.semaphore("gsem") as gsem,       # PE agg per-kj done
        nc.semaphore("vsem") as vsem,       # DVE epilogue per-kj done
        nc.semaphore("vv") as vv,           # DVE same-engine pipeline sync
        nc.semaphore("osem") as osem,       # out DMAs done
    ):
        @block.sync
        def _(sync):
            # drain detects DMA completion by polling the ring directly —
            # ~1us faster than the HWDGE completion-semaphore path
            sync.dma_start(out=xr[:], in_=xr_d[:]).then_inc(dsem_s, 16)
            # per-kj output DMAs fire as each epilogue chain lands, so the
            # kj0-2 stores overlap the remaining matmuls/epilogues
            for kj in range(KJ):
                sync.wait_ge(vsem, kj + 1)
                sync.dma_start(
                    out=out_d[kj * P:(kj + 1) * P, :], in_=o_sb[kj][:]
                ).then_inc(osem, 16)
            # no final osem wait: the Block-exit DRAIN on SP already blocks
            # until the DGE queues are empty, and the semaphore completion
            # path adds ~1.8us of latency on top of the actual transfer

        @block.gpsimd
        def _(gpsimd):
            gpsimd.dma_start(out=ydt[:], in_=ydt_d[:]).then_inc(dsem_g, 16)
            gpsimd.dma_start(out=wbb[:], in_=wbb_d[:]).then_inc(dsem_g, 16)

        @block.tensor
        def _(tensor):
            tensor.wait_ge(dsem_s, 16)
            for q in range(npair):
                g, p = divmod(q, NPAIR)
                if q >= 2:
                    tensor.wait_ge(asem, q - 1)  # A buffer q%2 free again
                for half in range(2):
                    ki = 2 * p + half
                    tensor.matmul(
                        A[q % 2][:, half * O_CORE:(half + 1) * O_CORE],
                        lxh[g][:, ki * P:(ki + 1) * P],
                        rth[g][:],
                        start=True,
                        stop=True,
                    ).then_inc(psem, 1)
            tensor.wait_ge(dsem_g, 32)  # ydt (all gpsimd-queue DMAs)
            n_mm = G * KI
            for kj in range(KJ):
                for g in range(G):
                    w = widths[g]
                    coff = 0 if g == 0 else 1
                    for ki in range(KI):
                        idx = g * KI + ki
                        q = g * NPAIR + ki // 2
                        if kj == 0 and ki % 2 == 0:
                            tensor.wait_ge(asem, q + 1)
                        rhs = ydt[:, KI * offs[g] + ki * w:
                                  KI * offs[g] + (ki + 1) * w]
                        lhs = E[q][:, (ki % 2) * O_CORE + kj * P:
                                   (ki % 2) * O_CORE + (kj + 1) * P]
                        mm = tensor.matmul(
                            pacc[kj][:, coff:coff + w],
                            lhs,
                            rhs,
                            start=(idx == 0),
                            stop=(idx == n_mm - 1),
                            skip_group_check=(G > 1),
                        )
                        if idx == n_mm - 1:
                            mm.then_inc(gsem, 1)

        @block.scalar
        def _(scalar):
            # touch Exp before the pipeline needs it: loads the ACT table
            # while the input DMAs are still in flight
            scalar.activation(warm[:], nc.const_aps.tensor(0.0, (1, 1)), Exp)
            for q in range(npair):
                scalar.wait_ge(psem, 2 * (q + 1))
                scalar.activation(E[q][:], A[q % 2][:], Exp).then_inc(asem, 1)

        @block.vector
        def _(vector):
            vector.wait_ge(dsem_g, 32)  # wbb resident
            # per-kj chain: runs as soon as that kj's PSUM bank is complete,
            # so kj0-2 finish during the remaining aggregation matmuls
            for kj in range(KJ):
                vector.wait_ge(gsem, kj + 1)
                vector.tensor_scalar_add(
                    denp[kj][:], pacc[kj][:, 0:1], EPS
                ).then_inc(vv, 1)
                vector.wait_ge(vv, 3 * kj + 1)  # denp through the pipe
                vector.scalar_tensor_tensor(
                    t1[kj][:], wb0[:], denp[kj][:], bt[:],
                    op0=mybir.AluOpType.mult, op1=mybir.AluOpType.add,
                ).then_inc(vv, 1)
                vector.reciprocal(recip[kj][:], denp[kj][:]).then_inc(vv, 1)
                vector.wait_ge(vv, 3 * kj + 3)  # t1 + recip through the pipe
                vector.scalar_tensor_tensor(
                    o_sb[kj][:], pacc[kj][:, 1:1 + OUT_CH], recip[kj][:], t1[kj][:],
                    op0=mybir.AluOpType.mult, op1=mybir.AluOpType.add,
                ).then_inc(vsem, 1)

    return nc


def _build_fp32(widths):
    """Fallback: fp32 rank-3 exponent matmul + fp32 aggregation (slower,
    used only when fp16 split values would overflow)."""
    import concourse.bacc as bacc
    import concourse.tile as tile
    from concourse import mybir

    f32 = mybir.dt.float32
    G = len(widths)
    wtot = sum(widths)
    offs = np.cumsum([0] + list(widths))

    nc = bacc.Bacc("TRN2", target_bir_lowering=False, debug=False)

    lx_d = nc.dram_tensor("lx", [3, N_IN], f32, kind="ExternalInput")
    rt_d = nc.dram_tensor("rt", [3 * G, O_CORE], f32, kind="ExternalInput")
    ydt_d = nc.dram_tensor("ydt", [P, KI * wtot], f32, kind="ExternalInput")
    wb0_d = nc.dram_tensor("wb0", [P, OUT_CH], f32, kind="ExternalInput")
    bt_d = nc.dram_tensor("bt", [P, OUT_CH], f32, kind="ExternalInput")
    out_d = nc.dram_tensor("out", [O_CORE, OUT_CH], f32, kind="ExternalOutput")

    with tile.TileContext(nc) as tc:
        with (
            tc.tile_pool(name="const", bufs=1) as cpool,
            tc.tile_pool(name="epool", bufs=1) as epool,
            tc.tile_pool(name="small", bufs=2) as spool,
            tc.tile_pool(name="outp", bufs=2) as opool,
            tc.tile_pool(name="apsum", bufs=3, space="PSUM") as apsum,
            tc.tile_pool(name="ppsum", bufs=1, space="PSUM") as ppsum,
        ):
            lx = cpool.tile([3, N_IN], f32, tag="lx")
            nc.sync.dma_start(lx[:], lx_d[:])
            rt = cpool.tile([3 * G, O_CORE], f32, tag="rt")
            nc.scalar.dma_start(rt[:], rt_d[:])
            ydt = cpool.tile([P, KI * wtot], f32, tag="ydt")
            nc.gpsimd.dma_start(ydt[:], ydt_d[:])
            wb0 = cpool.tile([P, OUT_CH], f32, tag="wb0")
            nc.gpsimd.dma_start(wb0[:], wb0_d[:])
            bt = cpool.tile([P, OUT_CH], f32, tag="bt")
            nc.gpsimd.dma_start(bt[:], bt_d[:])

            E = {}
            for g in range(G):
                for ki in range(KI):
                    a_ps = apsum.tile([P, O_CORE], f32, tag="A", name="a_ps")
                    nc.tensor.matmul(
                        a_ps[:],
                        lx[:, ki * P:(ki + 1) * P],
                        rt[3 * g:3 * g + 3, :],
                        start=True,
                        stop=True,
                    )
                    e = epool.tile([P, O_CORE], f32, tag=f"E{g}_{ki}", name="e")
                    nc.scalar.activation(
                        e[:], a_ps[:], mybir.ActivationFunctionType.Exp
                    )
                    E[(g, ki)] = e

            pacc = [
                ppsum.tile([P, 1 + OUT_CH], f32, tag=f"P{kj}", name=f"pacc{kj}")
                for kj in range(KJ)
            ]
            n_mm = G * KI
            for kj in range(KJ):
                for g in range(G):
                    w = widths[g]
                    coff = 0 if g == 0 else 1
                    for ki in range(KI):
                        idx = g * KI + ki
                        rhs = ydt[:, KI * offs[g] + ki * w: KI * offs[g] + (ki + 1) * w]
                        nc.tensor.matmul(
                            pacc[kj][:, coff:coff + w],
                            E[(g, ki)][:, kj * P:(kj + 1) * P],
                            rhs,
                            start=(idx == 0),
                            stop=(idx == n_mm - 1),
                            skip_group_check=(G > 1),
                        )

                denp = spool.tile([P, 1], f32, tag="denp", name="denp")
                nc.vector.tensor_scalar_add(denp[:], pacc[kj][:, 0:1], EPS)
                recip = spool.tile([P, 1], f32, tag="recip", name="recip")
                nc.vector.reciprocal(recip[:], denp[:])
                t1 = spool.tile([P, OUT_CH], f32, tag="t1", name="t1")
                nc.vector.scalar_tensor_tensor(
                    t1[:], wb0[:], denp[:], bt[:],
                    op0=mybir.AluOpType.mult, op1=mybir.AluOpType.add,
                )
                o_sb = opool.tile([P, OUT_CH], f32, tag="osb", name="o_sb")
                nc.vector.scalar_tensor_tensor(
                    o_sb[:], pacc[kj][:, 1:1 + OUT_CH], recip[:], t1[:],
                    op0=mybir.AluOpType.mult, op1=mybir.AluOpType.add,
                )
                nc.sync.dma_start(out_d[kj * P:(kj + 1) * P, :], o_sb[:])

    nc.compile()
    return nc


def _split2_f16(v):
    """2-way fp16 split: v ~= h1 + h2 with each half exactly fp16."""
    v = v.astype(np.float32)
    h1 = v.astype(np.float16)
    h2 = (v - h1.astype(np.float32)).astype(np.float16)
    return h1, h2


def _prepare_inputs(context_x, context_y, t, sigma, W, b):
    """Host prep: group channels by sigma, fold W, build per-core inputs."""
    sigma = np.asarray(sigma, dtype=np.float32)
    W64 = np.asarray(W, dtype=np.float64)
    b64 = np.asarray(b, dtype=np.float64)

    uniq = []
    for c in range(IN_CH):
        if sigma[c] not in uniq:
            uniq.append(sigma[c])
    uniq.sort(key=lambda s: (s != sigma[0]))  # channel-0 group first
    groups = [[c for c in range(IN_CH) if sigma[c] == s] for s in uniq]
    alphas = [0.5 / np.exp(2.0 * np.float64(s)) for s in uniq]
    widths = tuple((1 + OUT_CH) if 0 in g else OUT_CH for g in groups)
    G = len(groups)

    # fp16 path is safe unless sqrt(2a)*x or a*x^2 style terms overflow.
    xmax = max(
        float(np.abs(np.asarray(context_x)).max()),
        float(np.abs(np.asarray(t)).max()),
        1.0,
    )
    fp16_ok = all(a * xmax * xmax < 3e4 and np.isfinite(a) for a in alphas)

    in_maps = []
    for core in range(N_CORES):
        bidx, half = core // 2, core % 2
        x = np.asarray(context_x[bidx, :, 0], dtype=np.float64)
        th = np.asarray(t[bidx, half * O_CORE:(half + 1) * O_CORE, 0],
                        dtype=np.float64)
        y = np.asarray(context_y[bidx], dtype=np.float64)

        m = {}
        if fp16_ok:
            BLK = N_IN + O_CORE
            xr = np.empty((KEXP, G * BLK), dtype=np.float16)
            for g, a in enumerate(alphas):
                r = np.sqrt(2.0 * a)
                s1, s2 = _split2_f16(r * x)
                u1, u2 = _split2_f16(r * th)
                q1, q2 = _split2_f16(0.5 * (r * x) ** 2)
                w1, w2 = _split2_f16(0.5 * (r * th) ** 2)
                one_i = np.ones(N_IN, np.float16)
                neg1 = np.full(O_CORE, -1.0, np.float16)
                xr[:, g * BLK:g * BLK + N_IN] = np.stack(
                    [s1, s1, s2, s2, q1, q2, one_i, one_i]
                )
                xr[:, g * BLK + N_IN:(g + 1) * BLK] = np.stack(
                    [u1, u2, u1, u2, neg1, neg1, -w1, -w2]
                )
            m["xr"] = xr
        else:
            lx = np.stack([x, x * x, np.ones_like(x)]).astype(np.float32)
            rt = np.empty((3 * G, O_CORE), dtype=np.float32)
            for g, a in enumerate(alphas):
                rt[3 * g + 0] = 2.0 * a * th
                rt[3 * g + 1] = -a
                rt[3 * g + 2] = -a * th * th
            m["lx"], m["rt"] = lx, rt

        blocks = []
        for g, chans in enumerate(groups):
            w = widths[g]
            rhs = np.zeros((N_IN, w), dtype=np.float64)
            coff = 0
            if 0 in chans:
                rhs[:, 0] = 1.0
                coff = 1
            conv_ch = [c for c in chans if c > 0]
            if conv_ch:
                rhs[:, coff:] = y[:, [c - 1 for c in conv_ch]] @ W64[conv_ch, :]
            blocks.append(
                rhs.reshape(KI, P, w).transpose(1, 0, 2).reshape(P, KI * w)
            )
        ydt = np.concatenate(blocks, axis=1)
        m["ydt"] = ydt.astype(np.float16 if fp16_ok else np.float32)
        wb0 = np.tile(W64[0].astype(np.float32), (P, 1))
        bt = np.tile(b64.astype(np.float32), (P, 1))
        if fp16_ok:
            m["wbb"] = np.concatenate([wb0, bt], axis=1)
        else:
            m["wb0"], m["bt"] = wb0, bt
        in_maps.append(m)
    return widths, fp16_ok, in_maps


def _run(inputs: dict, trace: bool = False):
    """Compile (cached), run on 8 cores, gather. Returns (output, results)."""
    from concourse.bass_utils import run_bass_kernel_spmd

    widths, fp16_ok, in_maps = _prepare_inputs(
        inputs["context_x"], inputs["context_y"], inputs["t"],
        inputs["sigma"], inputs["W"], inputs["b"],
    )
    key = (widths, fp16_ok)
    if key not in _BASS_CACHE:
        _BASS_CACHE[key] = (_build_fp16_raw if fp16_ok else _build_fp32)(widths)
    nc = _BASS_CACHE[key]

    res = run_bass_kernel_spmd(nc, in_maps, list(range(N_CORES)), trace=trace)

    out = np.empty((B, N_OUT, OUT_CH), dtype=np.float32)
    for core in range(N_CORES):
        bidx, half = core // 2, core % 2
        out[bidx, half * O_CORE:(half + 1) * O_CORE, :] = res.results[core]["out"]
    return out, res


def kernel(**inputs) -> np.ndarray:
    out, _ = _run(inputs, trace=False)
    return out
